# revision 37
# baseline (speedup 1.0000x reference)
"""Causal self-attention (B=2, T=2048, dim=2048, 16 heads, RoPE) on 8 trn2
NeuronCores.

Sharding: core c handles batch b = c//4 and head group g = c%4 (4 heads each,
tensor-parallel over heads). Each core computes QKV projection + RoPE +
causal attention + its partial out-projection; the host sums the 4 partial
out-proj results per batch (the "all-reduce"), adds b_out, and stacks batches.

v3 design notes (all-bf16 pipeline):
  - Every matmul operand is bf16 (fp32 moving operands stream at ~2x the
    cycles and disable fast-weight-load; measured 390ns vs ~216ns per
    N=512 matmul). PSUM accumulation stays fp32.
  - Q/K/V live in SBUF between projection and attention (bf16 makes them
    6.3MB) -- no DRAM round-trip, no phase-transition stalls.
  - x and w_qkv are host-packed partition-major so every DMA moves 3-16KB
    contiguous per partition line (768B lines measured only ~200GB/s and
    starved the PE cold start for 30us).
  - QKV bias is fused into the PSUM->SBUF evacuation on the Act engine;
    out-proj bias is added on the host during the partial-sum reduction.
  - RoPE rotate-half stays a signed-permutation matmul; its PE ops for head
    h are emitted inside the next head's projection block so the PE never
    waits on the Act/DVE pipeline. Heads run in order [3,0,1,2] per t-slice
    so the final flush head (2) is consumed late in phase B.
  - Attention: softmax sums accumulate via a ones[128,128] stationary
    matmul, which broadcasts the sums across all partitions for free; DVE
    reciprocal_approx_fast reads them straight out of PSUM. exp runs on
    2-chunk PSUM tiles with a one-pair software pipeline so the PE never
    waits on the Act engine.
  - Phase C (out-proj) is interleaved per 512-wide query superblock: block
    sb's out-proj runs while attention for sb+1 computes; its PSUM->SBUF
    copies run on DVE (the Act engine is the co-bottleneck in phase B).
"""

import math
import os
import sys
import types

import numpy as np
import ml_dtypes

# ---------------------------------------------------------------------------
# NTFF profile hook (missing antenv.axon_hooks in this image). Reconstructed
# so run_bass_kernel_spmd(trace=True) can measure HW exec time.
# ---------------------------------------------------------------------------
try:
    import antenv

    if "antenv.axon_hooks" not in sys.modules:
        try:
            from trn_agent_boot.trn_boot import _ntff_profile_via_ctypes

            _hook = _ntff_profile_via_ctypes("/opt/axon/libaxon_pjrt.so")
        except Exception:
            _hook = None
        _m = types.ModuleType("antenv.axon_hooks")
        _m.get_axon_ntff_profile_hook = lambda: _hook
        _m.set_axon_ntff_profile_hook = lambda h: None
        sys.modules["antenv.axon_hooks"] = _m
        antenv.axon_hooks = _m
except Exception:
    pass

import concourse.bass as bass
import concourse.tile as tile
from concourse import bacc, mybir
from concourse.bass_utils import run_bass_kernel_spmd
from concourse.masks import make_identity

# Problem constants (hardcoded per the task contract).
B = 2
T = 2048
DIM = 2048
H = 16
HD = 128                  # head_dim
G = 4                     # head groups (cores per batch)
HPG = H // G              # heads per group = 4
N_CORES = 8
SCALE = 1.0 / math.sqrt(HD)

F32 = mybir.dt.float32
BF16 = mybir.dt.bfloat16
AF = mybir.ActivationFunctionType
NP_BF16 = ml_dtypes.bfloat16

TSL = 512                 # t-slice width in the projection phase
NTSL = T // TSL           # 4
QSB = 512                 # query super-block width in the attention phase
NSB = T // QSB            # 4
KC = 128                  # key chunk (partition dim)
NKCH = DIM // KC          # 16 contraction chunks in the projection
NJ = TSL // KC            # 4 v-transpose chunks per slice
NOB = DIM // 512          # 4 output-feature blocks in phase C

HEAD_ORDER = [3, 0, 1, 2]  # last processed head (2) is needed late in B

LAST_EXEC_NS = None
LAST_RESULTS = None

_PROGRAM_CACHE = {}


def _build_program():
    nc = bacc.Bacc("TRN2", target_bir_lowering=False, debug=False,
                   num_devices=N_CORES)

    x_pack = nc.dram_tensor("x_pack", [KC, NTSL, NKCH, TSL], BF16,
                            kind="ExternalInput").ap()
    w_pack = nc.dram_tensor("w_pack", [KC, HPG, NKCH, 3 * HD], BF16,
                            kind="ExternalInput").ap()
    b_cols = nc.dram_tensor("b_cols", [HD, 3 * HPG], F32,
                            kind="ExternalInput").ap()
    w_out = nc.dram_tensor("w_out_loc", [HPG * HD, DIM], BF16,
                           kind="ExternalInput").ap()
    cosT = nc.dram_tensor("cosT", [HD, T], BF16, kind="ExternalInput").ap()
    sinT = nc.dram_tensor("sinT", [HD, T], BF16, kind="ExternalInput").ap()
    permT = nc.dram_tensor("permT", [HD, HD], BF16, kind="ExternalInput").ap()
    masks = nc.dram_tensor("masks_t", [KC, QSB // KC, QSB], BF16,
                           kind="ExternalInput").ap()
    y = nc.dram_tensor("y_part", [T, DIM], BF16, kind="ExternalOutput").ap()

    with tile.TileContext(nc) as tc:
        _emit(tc, nc, x_pack, w_pack, b_cols, w_out, cosT, sinT, permT,
              masks, y)

    nc.compile()
    return nc


def _emit(tc, nc, x_pack, w_pack, b_cols_d, w_out, cosT_d, sinT_d, permT_d,
          masks_d, y):
    from contextlib import ExitStack

    ctx = ExitStack()
    with ctx:
        ctx.enter_context(nc.allow_low_precision(
            reason="bf16 matmul pipeline, fp32 PSUM accumulation"))

        # ---------------- persistent SBUF state -------------------------
        consts = ctx.enter_context(tc.tile_pool(name="consts", bufs=1))
        qkv = ctx.enter_context(tc.tile_pool(name="qkv", bufs=1))
        qtr = [qkv.tile([HD, T], BF16, tag=f"qtr{h}", name=f"qtr{h}")
               for h in range(HPG)]
        ktr = [qkv.tile([HD, T], BF16, tag=f"ktr{h}", name=f"ktr{h}")
               for h in range(HPG)]
        vh = [qkv.tile([KC, T // KC, HD], BF16, tag=f"vh{h}", name=f"vh{h}")
              for h in range(HPG)]

        # ======================= Phase A: QKV + RoPE ======================
        with (
            tc.tile_pool(name="a_w", bufs=1) as a_w,
            tc.tile_pool(name="a_x", bufs=3) as a_x,
            tc.tile_pool(name="a_sb", bufs=3) as a_sb,
            tc.tile_pool(name="a_ps", bufs=6, space="PSUM") as a_ps,
            tc.tile_pool(name="a_ps2", bufs=1, space="PSUM") as a_ps2,
            tc.tile_pool(name="a_pst", bufs=1, space="PSUM") as a_pst,
        ):
            # Cold-start DMA order: x slice 0 and the first head's weights
            # gate the first matmuls; everything else after. Packed layouts
            # give 3-16KB contiguous per partition line. Dependencies are
            # tile-granular, so the cold-start path uses 4 separate tiles
            # per x-slice / weight-head: the first matmul only waits for the
            # first ~0.9MB instead of the full 3.7MB.
            NSPL = 4
            CSTEP = NKCH // NSPL
            xsl_tiles = {}

            def load_xsl(tsl):
                xt = [a_x.tile([KC, CSTEP, TSL], BF16, tag=f"xsl{jj}",
                               name=f"xsl{tsl}_{jj}") for jj in range(NSPL)]
                for jj in range(NSPL):
                    nc.sync.dma_start(
                        out=xt[jj],
                        in_=x_pack[:, tsl, jj * CSTEP:(jj + 1) * CSTEP, :])
                xsl_tiles[tsl] = xt

            w_t = {}

            def load_w(h):
                for jj in range(NSPL):
                    wt = a_w.tile([KC, CSTEP, 3 * HD], BF16,
                                  tag=f"w{h}_{jj}")
                    nc.sync.dma_start(
                        out=wt,
                        in_=w_pack[:, h, jj * CSTEP:(jj + 1) * CSTEP, :])
                    w_t[(h, jj)] = wt

            # interleave the first head's weight DMAs with x slice 0 so the
            # earliest-needed regions land on the first (parallel) queues;
            # non-DMA setup ops between them split the sync engine's DMA
            # trigger groups so the first matmul only waits the first pair.
            ident = consts.tile([KC, KC], BF16, tag="ident")
            ones_f32 = consts.tile([KC, KC], F32, tag="ones_f32")
            ones_sq = consts.tile([KC, KC], BF16, tag="ones_sq")
            xt0 = [a_x.tile([KC, CSTEP, TSL], BF16, tag=f"xsl{jj}",
                            name=f"xsl0_{jj}") for jj in range(NSPL)]
            seps = [
                lambda: nc.vector.memset(ones_f32, 1.0),
                lambda: nc.gpsimd.memset(ident, 0.0),
                lambda: nc.vector.tensor_copy(ones_sq, ones_f32),
                lambda: make_identity(nc, ident, nomemset=True),
            ]
            for jj in range(NSPL):
                nc.sync.dma_start(
                    out=xt0[jj], in_=x_pack[:, 0, jj * CSTEP:(jj + 1) * CSTEP, :])
                wt = a_w.tile([KC, CSTEP, 3 * HD], BF16,
                              tag=f"w{HEAD_ORDER[0]}_{jj}")
                nc.sync.dma_start(
                    out=wt,
                    in_=w_pack[:, HEAD_ORDER[0], jj * CSTEP:(jj + 1) * CSTEP, :])
                w_t[(HEAD_ORDER[0], jj)] = wt
                seps[jj]()
            xsl_tiles[0] = xt0
            for h in HEAD_ORDER[1:]:
                load_w(h)
            cosT = consts.tile([HD, T], BF16, tag="cosT")
            nc.sync.dma_start(out=cosT, in_=cosT_d)
            sinT = consts.tile([HD, T], BF16, tag="sinT")
            nc.sync.dma_start(out=sinT, in_=sinT_d)
            permT = consts.tile([HD, HD], BF16, tag="permT")
            nc.sync.dma_start(out=permT, in_=permT_d)
            bcols = consts.tile([HD, 3 * HPG], F32, tag="bcols")
            nc.sync.dma_start(out=bcols, in_=b_cols_d)
            mask_t = consts.tile([KC, QSB // KC, QSB], BF16, tag="masks")
            nc.sync.dma_start(out=mask_t, in_=masks_d)

            # pending = [tsl, h, qb_q, qb_k, vb, aux]; its RoPE/transpose PE
            # ops are interleaved into the NEXT head's projection block (so
            # the PE never stalls on Act/DVE latency).
            pending = None

            def emit_pending_psr(kind):
                psr = a_ps2.tile([HD, TSL], F32, tag="ps_rot",
                                 name=f"psr_{pending[0]}_{pending[1]}_{kind}")
                nc.tensor.matmul(psr, permT, pending[2 + kind],
                                 start=True, stop=True)
                pending[5][kind] = psr

            def emit_pending_transposes():
                tsl, h = pending[0], pending[1]
                pstT = a_pst.tile([KC, NJ, KC], BF16, tag="ps_t",
                                  name=f"pst_{tsl}_{h}")
                for j in range(NJ):
                    nc.tensor.transpose(pstT[:, j, :],
                                        pending[4][:, j * KC:(j + 1) * KC],
                                        ident)
                pending[5][2] = pstT

            def emit_pending_dve():
                tsl, h, qb_q, qb_k, vb, aux = pending
                t0 = tsl * TSL
                for kind, qb, dst in ((0, qb_q, qtr), (1, qb_k, ktr)):
                    psr = aux[kind]
                    m1 = a_sb.tile([HD, TSL], BF16, tag="m1")
                    nc.vector.tensor_mul(m1, qb, cosT[:, t0:t0 + TSL])
                    m2 = a_sb.tile([HD, TSL], BF16, tag="m2")
                    nc.vector.tensor_mul(m2, psr, sinT[:, t0:t0 + TSL])
                    nc.vector.tensor_add(dst[h][:, t0:t0 + TSL], m1, m2)
                nc.vector.tensor_copy(
                    vh[h][:, tsl * NJ:(tsl + 1) * NJ, :], aux[2])

            for tsl in range(NTSL):
                if tsl + 1 < NTSL:
                    load_xsl(tsl + 1)
                xsl = xsl_tiles[tsl]
                for h in HEAD_ORDER:
                    ps = [a_ps.tile([HD, TSL], F32, tag="ps_qkv",
                                    name=f"ps_{tsl}_{h}_{k}")
                          for k in range(3)]
                    if pending is None:
                        # very first block: kind-outer order finishes the q
                        # accumulation first so the Act engine (and the RoPE
                        # chain behind it) gets a ~7us head start on the
                        # cold-started PE.
                        for kind in range(3):
                            for kc in range(NKCH):
                                nc.tensor.matmul(
                                    ps[kind],
                                    w_t[(h, kc // CSTEP)][:, kc % CSTEP,
                                                          kind * HD:(kind + 1) * HD],
                                    xsl[kc // CSTEP][:, kc % CSTEP, :],
                                    start=(kc == 0), stop=(kc == NKCH - 1),
                                )
                    else:
                        for kc in range(NKCH):
                            for kind in range(3):
                                nc.tensor.matmul(
                                    ps[kind],
                                    w_t[(h, kc // CSTEP)][:, kc % CSTEP,
                                                          kind * HD:(kind + 1) * HD],
                                    xsl[kc // CSTEP][:, kc % CSTEP, :],
                                    start=(kc == 0), stop=(kc == NKCH - 1),
                                )
                            if kc == 5:
                                emit_pending_psr(0)
                            elif kc == 9:
                                emit_pending_psr(1)
                            elif kc == 13:
                                emit_pending_transposes()
                    # evacuate PSUM via Act (fused +bias, bf16 out)
                    qb_q = a_sb.tile([HD, TSL], BF16, tag="qb_q")
                    nc.scalar.activation(qb_q, ps[0], AF.Identity,
                                         bias=bcols[:, h:h + 1])
                    qb_k = a_sb.tile([HD, TSL], BF16, tag="qb_k")
                    nc.scalar.activation(qb_k, ps[1], AF.Identity,
                                         bias=bcols[:, HPG + h:HPG + h + 1])
                    vb = a_sb.tile([HD, TSL], BF16, tag="vb")
                    nc.scalar.activation(vb, ps[2], AF.Identity,
                                         bias=bcols[:, 2 * HPG + h:2 * HPG + h + 1])
                    if pending is not None:
                        emit_pending_dve()
                    pending = [tsl, h, qb_q, qb_k, vb, [None, None, None]]

            # flush the last head (HEAD_ORDER[-1]); its DVE tail overlaps
            # the start of phase B (that head is needed third there).
            emit_pending_psr(0)
            emit_pending_psr(1)
            emit_pending_transposes()
            emit_pending_dve()
            pending = None

        # Phase C weights: loaded during phase B.
        c_w = ctx.enter_context(tc.tile_pool(name="c_w", bufs=1))
        wo = c_w.tile([KC, HPG, DIM], BF16, tag="wo")
        w_out_r = w_out.rearrange("(c p) o -> p c o", p=KC)
        for hc in range(HPG):
            nc.sync.dma_start(out=wo[:, hc, :], in_=w_out_r[:, hc, :])

        # ================== Phase B + C: attention, out-proj ==============
        b_ot = ctx.enter_context(tc.tile_pool(name="b_ot", bufs=3))
        c_sb = ctx.enter_context(tc.tile_pool(name="c_sb", bufs=4))

        def emit_outproj(sb, ot_sb, c_ps, last):
            # out-proj for query superblock sb (t rows sb*512..+512).
            # PSUM->SBUF copies stay on DVE (mixing engines here causes
            # head-of-line blocking on the sync DMA-trigger queue). A
            # full 128x2048 row block goes out as one DMA (4KB lines).
            for tb in range(QSB // KC):
                tt0 = sb * QSB + tb * KC
                ysr = c_sb.tile([KC, DIM], BF16, tag="ysr")
                for ob in range(NOB):
                    o0 = ob * 512
                    ps_y = c_ps.tile([KC, 512], F32, tag="ps_y")
                    for hc in range(HPG):
                        nc.tensor.matmul(
                            ps_y,
                            ot_sb[hc][:, tb * KC:(tb + 1) * KC],
                            wo[:, hc, o0:o0 + 512],
                            start=(hc == 0), stop=(hc == HPG - 1),
                        )
                    nc.vector.tensor_copy(ysr[:, o0:o0 + 512], ps_y)
                if last and tb % 2 == 1:
                    # final superblock: split the 2MB drain across two
                    # DMA paths so the tail isn't one serialized queue.
                    nc.gpsimd.dma_start(out=y[tt0:tt0 + KC, :], in_=ysr)
                else:
                    nc.sync.dma_start(out=y[tt0:tt0 + KC, :], in_=ysr)

        with (
            tc.tile_pool(name="b_pt", bufs=3) as b_pt,
            tc.tile_pool(name="b_sm", bufs=3) as b_sm,
            tc.tile_pool(name="b_ps_s", bufs=2, space="PSUM") as b_ps_s,
            tc.tile_pool(name="b_ps_o", bufs=1, space="PSUM") as b_ps_o,
            tc.tile_pool(name="b_ps_sum", bufs=1, space="PSUM") as b_ps_sum,
            tc.tile_pool(name="c_ps", bufs=2, space="PSUM") as c_ps,
        ):

            prev = None                      # (sb, ot_sb) awaiting out-proj
            # sb=2 first: its 6-pair prologue covers the exp latency right
            # after phase A; all-diagonal sb=0 runs clear of the flush tail.
            for sb in (2, 0, 1, 3):
                q0 = sb * QSB
                nk = (sb + 1) * (QSB // KC)  # causal key chunks
                npair = nk // 2
                ot_sb = []
                for h in range(HPG):
                    ps_o = b_ps_o.tile([HD, QSB], F32, tag="ps_o")
                    # ones[128,128] stationary: sums replicated across all
                    # partitions (free broadcast for the normalization).
                    ps_sum = b_ps_sum.tile([KC, QSB], F32, tag="ps_sum")
                    ps_s_tiles = {}

                    # diagonal chunks (dj >= 0) only produce output for
                    # query columns >= dj*128; everything below is masked
                    # out, so S/exp/O/sum are narrowed to [dj*128, 512).
                    # sb=0 is all-diagonal: narrowing there leaves the PE
                    # with too little work to hide Act/DVE latency, so it
                    # stays on the full-width pair path.
                    def col0(kci):
                        if nk == QSB // KC:
                            return 0
                        dj = kci - (nk - QSB // KC)
                        return max(dj, 0) * KC

                    def emit_s_pair(gp):
                        ps_s = b_ps_s.tile([KC, 2, QSB], F32, tag="ps_st")
                        for j in range(2):
                            kci = 2 * gp + j
                            c0 = col0(kci)
                            nc.tensor.matmul(
                                ps_s[:, j, c0:],
                                ktr[h][:, kci * KC:(kci + 1) * KC],
                                qtr[h][:, q0 + c0:q0 + QSB],
                                start=True, stop=True,
                            )
                        ps_s_tiles[gp] = ps_s

                    emit_s_pair(0)
                    nsum = 0
                    for gp in range(npair):
                        ps_s = ps_s_tiles.pop(gp)
                        pt = b_pt.tile([KC, 2, QSB], BF16, tag="pt")
                        diag = col0(2 * gp + 1) > 0
                        if diag or gp == 0:
                            # per-chunk exp: on the first pair of a head it
                            # halves the S->exp->O latency (the O-matmul
                            # otherwise stalls ~0.9us with nothing to cover
                            # it); on diagonal pairs it skips masked columns.
                            for j in range(2):
                                c0 = col0(2 * gp + j)
                                nc.scalar.activation(
                                    pt[:, j, c0:], ps_s[:, j, c0:], AF.Exp,
                                    scale=SCALE)
                        else:
                            nc.scalar.activation(pt, ps_s, AF.Exp,
                                                 scale=SCALE)
                        if gp + 1 < npair:
                            emit_s_pair(gp + 1)
                        for j in range(2):
                            dj = 2 * gp + j - (nk - QSB // KC)
                            if dj >= 0:
                                c0 = col0(2 * gp + j)
                                nc.vector.tensor_mul(
                                    pt[:, j, c0:], pt[:, j, c0:],
                                    mask_t[:, dj, c0:])
                        for j in range(2):
                            kci = 2 * gp + j
                            c0 = col0(kci)
                            nc.tensor.matmul(
                                ps_o[:, c0:], vh[h][:, kci, :],
                                pt[:, j, c0:],
                                start=(kci == 0), stop=(kci == nk - 1),
                            )
                        if diag:
                            # narrowed per-chunk softmax-sum matmuls
                            for j in range(2):
                                kci = 2 * gp + j
                                c0 = col0(kci)
                                nc.tensor.matmul(
                                    ps_sum[:, c0:], ones_sq, pt[:, j, c0:],
                                    start=(nsum == 0), stop=(kci == nk - 1),
                                )
                                nsum += 1
                        else:
                            # pair-add the probabilities on DVE so only one
                            # ones-matmul per pair feeds the softmax sums.
                            ptsum = b_pt.tile([KC, QSB], BF16, tag="ptsum")
                            nc.vector.tensor_add(ptsum, pt[:, 0, :],
                                                 pt[:, 1, :])
                            nc.tensor.matmul(
                                ps_sum, ones_sq, ptsum,
                                start=(nsum == 0), stop=(2 * gp + 1 == nk - 1),
                            )
                            nsum += 1
                    # normalize: ot = ps_o * (1 / sums), sums already
                    # broadcast across partitions by the ones matmul.
                    rb = b_sm.tile([KC, QSB], F32, tag="rb")
                    nc.vector.reciprocal_approx_fast(rb, ps_sum)
                    ot = b_ot.tile([HD, QSB], BF16, tag=f"ot{h}",
                                   name=f"ot_{sb}_{h}")
                    nc.vector.tensor_mul(ot, ps_o, rb)
                    ot_sb.append(ot)
                if prev is not None:
                    emit_outproj(*prev, c_ps=c_ps, last=False)
                prev = (sb, ot_sb)
        # final superblock's out-proj: the attention PSUM pools are done, so
        # it gets a deeper PSUM ring (no per-group wait on the copy 2 back).
        with tc.tile_pool(name="c_ps2", bufs=4, space="PSUM") as c_ps2:
            emit_outproj(*prev, c_ps=c_ps2, last=True)


# ---------------------------------------------------------------------------
# Host-side input prep
# ---------------------------------------------------------------------------


def _rope_tables():
    inv_freq = 1.0 / (10000.0 ** (np.arange(0, HD, 2, dtype=np.float64) / HD))
    t = np.arange(T, dtype=np.float64)
    freqs = np.outer(t, inv_freq)                     # [T, 64]
    emb = np.concatenate([freqs, freqs], axis=-1)     # [T, 128]
    cosT = np.cos(emb).T.astype(NP_BF16)              # [128, T]
    sinT = np.sin(emb).T.astype(NP_BF16)
    return np.ascontiguousarray(cosT), np.ascontiguousarray(sinT)


def _perm_signed_T():
    p = np.zeros((HD, HD), dtype=np.float32)
    half = HD // 2
    for dp in range(half):
        p[dp, dp + half] = -1.0
    for dp in range(half, HD):
        p[dp, dp - half] = 1.0
    return np.ascontiguousarray(p.T.astype(NP_BF16))


def _masks_t():
    # masks[r, j, c] = 1 if c >= j*128 + r  (causal mask for the diagonal
    # 512-wide block, per 128-key chunk j)
    r = np.arange(KC)[:, None, None]
    j = np.arange(QSB // KC)[None, :, None]
    c = np.arange(QSB)[None, None, :]
    return (c >= j * KC + r).astype(NP_BF16)


def kernel(x, w_qkv, b_qkv, w_out, b_out):
    global LAST_EXEC_NS, LAST_RESULTS

    x = np.asarray(x, dtype=np.float32)
    w_qkv = np.asarray(w_qkv, dtype=np.float32)
    b_qkv = np.asarray(b_qkv, dtype=np.float32)
    w_out = np.asarray(w_out, dtype=np.float32)
    b_out = np.asarray(b_out, dtype=np.float32)

    if "prog" not in _PROGRAM_CACHE:
        _PROGRAM_CACHE["prog"] = _build_program()
    nc = _PROGRAM_CACHE["prog"]

    cosT, sinT = _rope_tables()
    permT = _perm_signed_T()
    masks = _masks_t()

    # x packed partition-major: [p, tsl, c, tl] = xT[c*128+p, tsl*512+tl]
    xps = []
    for b in range(B):
        xT = np.ascontiguousarray(x[b].T).astype(NP_BF16)
        xp = xT.reshape(NKCH, KC, NTSL, TSL).transpose(1, 2, 0, 3)
        xps.append(np.ascontiguousarray(xp))

    in_maps = []
    for c in range(N_CORES):
        b = c // G
        g = c % G
        f0 = g * HPG * HD
        # head-major packing: [h][q(128), k(128), v(128)] then
        # partition-major: [p, h, c, f] = w_loc[c*128+p, h*384+f]
        w_loc = np.concatenate(
            [w_qkv[:, base + f0 + h * HD: base + f0 + (h + 1) * HD]
             for h in range(HPG)
             for base in (0, DIM, 2 * DIM)], axis=1).astype(NP_BF16)
        w_pack = np.ascontiguousarray(
            w_loc.reshape(NKCH, KC, HPG, 3 * HD).transpose(1, 2, 0, 3))
        b_loc = np.concatenate(
            [b_qkv[f0:f0 + HPG * HD], b_qkv[DIM + f0:DIM + f0 + HPG * HD],
             b_qkv[2 * DIM + f0:2 * DIM + f0 + HPG * HD]])
        b_cols = np.ascontiguousarray(
            b_loc.reshape(3 * HPG, HD).T).astype(np.float32)
        w_out_loc = np.ascontiguousarray(
            w_out[f0:f0 + HPG * HD, :].astype(NP_BF16))
        in_maps.append({
            "x_pack": xps[b],
            "w_pack": w_pack,
            "b_cols": b_cols,
            "w_out_loc": w_out_loc,
            "cosT": cosT,
            "sinT": sinT,
            "permT": permT,
            "masks_t": masks,
        })

    trace = bool(os.environ.get("BASS_KERNEL_TRACE"))
    res = run_bass_kernel_spmd(nc, in_maps, list(range(N_CORES)), trace=trace)
    LAST_EXEC_NS = res.exec_time_ns
    LAST_RESULTS = res

    out = np.empty((B, T, DIM), dtype=np.float32)
    for b in range(B):
        acc = res.results[4 * b]["y_part"].astype(np.float32)
        for g in range(1, G):
            acc = acc + res.results[4 * b + g]["y_part"].astype(np.float32)
        out[b] = acc + b_out[None, :]
    return out
